# revision 11
# baseline (speedup 1.0000x reference)
"""Attention-head decoder kernel for Trainium2 (8 NeuronCores).

Full on-device implementation, data-parallel over batch (B=512 -> 64/core).
Per core, everything runs in one Bass/Tile NEFF:
  1. Encoder projection bhpT[h,(s,b)] = w_i2h @ batch_H^T (PE, bf16).
  2. 26-step attention/LSTM recurrence, batch split into 2 groups of 32
     that interleave so one group's big tanh pipeline (ACT) overlaps the
     other group's context/gates matmuls (PE).
  3. Output projection probs = h @ w_gen^T per step.

Layouts (per core, b = local batch 0..64, groups g of 32):
  bhpT   [128p=h-chunk, (hc,4)(s,128)(b,32|g)]  encoder projection, bf16
  hs     [128p=s, (b,64)(d,512)]                batch_H s-major, bf16
  hidT   [128p=h-chunk, (hc,4)(b,32)]           per-step attention hidden
  e      psum [8p=s-piece, (s_lo,16)(b,32)]     scores via wsc-diag stationary
  expT   [128p=s, (b,32)]                       de-interleaved via DMA
  ctx    psum [32p=b, (d,512)]                  via exp-diag stationary mms
  gates  psum [32p=b, (gate,512)] x4 quarters   i,f,g,o
Softmax skips the max-subtraction (|e| <= ~6, exp is safe in fp32/bf16)
and folds the denominator into a reciprocal scale on the context copy.
sigmoid(x) = 0.5*tanh(0.5x)+0.5 keeps ACT on one table set (exp/tanh).

Hardcoded: B=512, S=128, D=512, H=512, C=38, T=26, 8 cores.
"""

import numpy as np
from contextlib import ExitStack

B, S, D, H, C, T = 512, 128, 512, 512, 38, 26
NCORES = 8
BL = B // NCORES            # 64 batch per core
NG = 2                      # batch groups per core
BG = BL // NG               # 32 batch per group
NHC = 4                     # 128-row chunks of H (and D)
M = S * BL                  # 8192 = (s,b) columns per core

_cache = {}


def _build_bass():
    import concourse.bass as bass
    import concourse.mybir as mybir
    from concourse.bass import AP
    from concourse.bacc import Bacc
    from concourse.tile import TileContext

    bf = mybir.dt.bfloat16
    f32 = mybir.dt.float32
    AF = mybir.ActivationFunctionType
    OP = mybir.AluOpType

    nc = Bacc()

    # ---- DRAM I/O ----
    bht = nc.dram_tensor("bht", [D, M], bf, kind="ExternalInput")          # (d, s*64+b)
    hs = nc.dram_tensor("hs", [S, BL * D], bf, kind="ExternalInput")      # (s, b*512+d)
    wi2ht = nc.dram_tensor("wi2ht", [D, H], bf, kind="ExternalInput")     # w_i2h.T
    wh2ht = nc.dram_tensor("wh2ht", [H, H], bf, kind="ExternalInput")     # w_h2h.T
    wiht = nc.dram_tensor("wiht", [D, 4 * H], bf, kind="ExternalInput")   # w_ih[:, :D].T
    wihaug = nc.dram_tensor("wihaug", [C + 1, 4 * H], bf, kind="ExternalInput")
    whht = nc.dram_tensor("whht", [H, 4 * H], bf, kind="ExternalInput")   # w_hh.T
    wgent = nc.dram_tensor("wgent", [H, C], bf, kind="ExternalInput")     # w_gen.T
    bgen = nc.dram_tensor("bgen", [1, C], bf, kind="ExternalInput")
    wscst = nc.dram_tensor("wscst", [128, NHC * 8 * 8], bf, kind="ExternalInput")
    onehot = nc.dram_tensor("onehot", [C + 1, T * BL], bf, kind="ExternalInput")
    bh2ht = nc.dram_tensor("bh2ht", [128, NHC], f32, kind="ExternalInput")
    ident = nc.dram_tensor("ident", [BG, BG], bf, kind="ExternalInput")
    ones1 = nc.dram_tensor("ones1", [128, 1], bf, kind="ExternalInput")
    onest = nc.dram_tensor("onest", [1, BG], bf, kind="ExternalInput")
    out = nc.dram_tensor("out", [BL, T * C], f32, kind="ExternalOutput")

    def bview(ap, newap, extra_off=0):
        return AP(ap.tensor, ap.offset + extra_off, newap)

    with TileContext(nc) as tc, ExitStack() as ctx:
        cst = ctx.enter_context(tc.tile_pool(name="cst", bufs=1))

        c_hs = cst.tile([S, BL * D], bf)
        nc.sync.dma_start(c_hs[:], hs[:])
        c_wh2h = cst.tile([128, NHC * H], bf)
        nc.sync.dma_start(
            c_wh2h[:].rearrange("p (k h) -> p k h", k=NHC),
            wh2ht[:].rearrange("(k p) h -> p k h", p=128))
        c_wih = cst.tile([128, NHC * 4 * H], bf)
        nc.sync.dma_start(
            c_wih[:].rearrange("p (k g) -> p k g", k=NHC),
            wiht[:].rearrange("(k p) g -> p k g", p=128))
        c_whh = cst.tile([128, NHC * 4 * H], bf)
        nc.sync.dma_start(
            c_whh[:].rearrange("p (k g) -> p k g", k=NHC),
            whht[:].rearrange("(k p) g -> p k g", p=128))
        c_wihaug = cst.tile([C + 1, 4 * H], bf)
        nc.sync.dma_start(c_wihaug[:], wihaug[:])
        c_wgen = cst.tile([128, NHC * C], bf)
        nc.sync.dma_start(
            c_wgen[:].rearrange("p (k c) -> p k c", k=NHC),
            wgent[:].rearrange("(k p) c -> p k c", p=128))
        c_bgen = cst.tile([1, C], bf)
        nc.sync.dma_start(c_bgen[:], bgen[:])
        c_wsc = cst.tile([128, NHC * 8 * 8], bf)
        nc.sync.dma_start(c_wsc[:], wscst[:])
        c_bh2h = cst.tile([128, NHC], f32)
        nc.sync.dma_start(c_bh2h[:], bh2ht[:])
        c_ident = cst.tile([BG, BG], bf)
        nc.sync.dma_start(c_ident[:], ident[:])
        c_ones1 = cst.tile([128, 1], bf)
        nc.sync.dma_start(c_ones1[:], ones1[:])
        c_onest = cst.tile([1, BG], bf)
        nc.sync.dma_start(c_onest[:], onest[:])

        bhpT = cst.tile([128, NHC * M], bf)          # (hc)(s)(b)
        diag = [cst.tile([128, BG * BG], bf, name=f"diag{_g}") for _g in range(NG)]
        for g in range(NG):
            nc.vector.memset(diag[g][:], 0.0)
        # persistent cell state (f32) + initial zero hT
        c_st = [cst.tile([BG, H], f32, name=f"cstate{_g}") for _g in range(NG)]
        hT0 = [cst.tile([128, NHC * BG], bf, name=f"hT0{_g}") for _g in range(NG)]
        for g in range(NG):
            nc.vector.memset(c_st[g][:], 0.0)
            nc.vector.memset(hT0[g][:], 0.0)

        # ---- Phase 1: encoder projection ----
        NP = 32          # 256-column pieces of M
        PW = M // NP     # 256
        with tc.tile_pool(name="proj", bufs=2) as pj, \
             tc.tile_pool(name="pswi", bufs=1) as pw, \
             tc.tile_pool(name="psproj", bufs=2, space="PSUM") as psp:
            c_wi2h = pw.tile([128, NHC * H], bf)
            nc.sync.dma_start(
                c_wi2h[:].rearrange("p (k h) -> p k h", k=NHC),
                wi2ht[:].rearrange("(k p) h -> p k h", p=128))
            for j in range(NP):
                piece = pj.tile([128, NHC * PW], bf, tag="piece")
                nc.gpsimd.dma_start(
                    piece[:].rearrange("p (k m) -> p k m", k=NHC),
                    bht[:, j * PW:(j + 1) * PW].rearrange("(k p) m -> p k m", p=128))
                for hc in range(NHC):
                    acc = psp.tile([128, PW], f32, tag="acc")
                    for dc in range(NHC):
                        nc.tensor.matmul(
                            acc[:],
                            c_wi2h[:, dc * H + hc * 128: dc * H + (hc + 1) * 128],
                            piece[:, dc * PW:(dc + 1) * PW],
                            start=(dc == 0), stop=(dc == NHC - 1))
                    dst = bhpT[:, hc * M + j * PW: hc * M + (j + 1) * PW]
                    if (j * NHC + hc) % 2 == 0:
                        nc.vector.tensor_copy(dst, acc[:])
                    else:
                        nc.scalar.copy(dst, acc[:])

        # ---- Phase 2: recurrence ----
        TP = 1024        # tanh piece columns (s-major, b=32 minor)
        NTP = (S * BG) // TP   # 4 pieces per (g, hc)
        JP = 8           # score pieces per group (16 s x 32 b = 512 cols each)
        JW = S * BG // JP      # 512

        stp = ctx.enter_context(tc.tile_pool(name="stp", bufs=2))
        drp = ctx.enter_context(tc.tile_pool(name="drp", bufs=2, space="DRAM"))
        big = ctx.enter_context(tc.tile_pool(name="big", bufs=2))
        ps_e = ctx.enter_context(tc.tile_pool(name="ps_e", bufs=2, space="PSUM"))
        ps_mm = ctx.enter_context(tc.tile_pool(name="ps_mm", bufs=4, space="PSUM"))
        ps_sm = ctx.enter_context(tc.tile_pool(name="ps_sm", bufs=2, space="PSUM"))

        hT_prev = [hT0[0], hT0[1]]
        front_st = [None, None]
        oh_cur = [None]

        def front(g, t):
            """hid matmuls + broadcast-add + tanh + score matmuls for group g."""
            if g == 0:
                oh_t = stp.tile([C + 1, BL], bf, tag="oh")
                nc.gpsimd.dma_start(oh_t[:], onehot[:, t * BL:(t + 1) * BL])
                oh_cur[0] = oh_t
            hTp = hT_prev[g]
            # hid: hidT[hc*128+p, b] = sum_k w_h2h[h,k] h[b,k] + b_h2h
            ph = ps_sm.tile([128, NHC * BG], f32, tag="sm")
            for hc in range(NHC):
                for kc in range(NHC):
                    nc.tensor.matmul(
                        ph[:, hc * BG:(hc + 1) * BG],
                        c_wh2h[:, kc * H + hc * 128: kc * H + (hc + 1) * 128],
                        hTp[:, kc * BG:(kc + 1) * BG],
                        start=(kc == 0), stop=(kc == NHC - 1))
            hidT = stp.tile([128, NHC * BG], bf, tag=f"hidT{g}")
            for hc in range(NHC):
                nc.scalar.activation(
                    hidT[:, hc * BG:(hc + 1) * BG], ph[:, hc * BG:(hc + 1) * BG],
                    AF.Identity, bias=c_bh2h[:, hc:hc + 1])
            # big pipeline: add (DVE, bcast) -> tanh (ACT) -> score mms (PE)
            pe = ps_e.tile([8, JP * 8 * 8], f32, tag="e")  # [8, 512]
            first = True
            for hc in range(NHC):
                for tp in range(NTP):
                    srows = TP // BG          # s-values per piece (64)
                    s0 = tp * srows
                    tmp = big.tile([128, TP], bf, tag="tmp")
                    a_in0 = bview(bhpT[:], [[NHC * M, 128], [BL, srows], [1, BG]],
                                  hc * M + s0 * BL + g * BG)
                    a_in1 = bview(hidT[:], [[NHC * BG, 128], [0, srows], [1, BG]],
                                  hc * BG)
                    a_out = tmp[:].rearrange("p (s b) -> p s b", b=BG)
                    nc.vector.tensor_tensor(a_out, a_in0, a_in1, OP.add)
                    tb = big.tile([128, TP], bf, tag="tb")
                    nc.scalar.activation(tb[:], tmp[:], AF.Tanh)
                    jpp = TP // JW            # score pieces in this tanh piece (4)
                    for jj in range(jpp):
                        j = tp * jpp + jj
                        last = (hc == NHC - 1) and (j == JP - 1)
                        nc.tensor.matmul(
                            pe[:],
                            c_wsc[:, (hc * 8 + j) * 8:(hc * 8 + j) * 8 + 8],
                            tb[:, jj * JW:(jj + 1) * JW],
                            start=first, stop=last)
                        first = False
            front_st[g] = (pe,)

        def post(g, t):
            """softmax fold + context + gates + LSTM + output for group g."""
            (pe,) = front_st[g]
            # exp, de-interleave to s-partition layout, diag scatter
            exp_sb = stp.tile([8, JW], bf, tag="exp")
            nc.scalar.activation(exp_sb[:], pe[:], AF.Exp)
            scr = drp.tile([JP, JW], bf, tag="scr")
            nc.gpsimd.dma_start(scr[:], exp_sb[:])
            expT = stp.tile([128, BG], bf, tag="expT")
            nc.gpsimd.dma_start(
                expT[:], scr[:].rearrange("j (sl b) -> (j sl) b", sl=16))
            dst = bview(diag[g][:], [[BG * BG, 128], [BG + 1, BG]])
            nc.vector.tensor_copy(dst, expT[:])
            # denominator + reciprocal
            pden = ps_sm.tile([BG, 1], f32, tag="sm")
            nc.tensor.matmul(pden[:], expT[:], c_ones1[:], start=True, stop=True)
            recip = stp.tile([BG, 1], f32, tag="recip")
            nc.vector.reciprocal(recip[:], pden[:])
            # context: per-b diag stationary, accumulate rows
            pctx = ps_mm.tile([BG, D], f32, tag="mm32")
            for b in range(BG):
                nc.tensor.matmul(
                    pctx[:],
                    diag[g][:, b * BG:(b + 1) * BG],
                    c_hs[:, (g * BG + b) * D:(g * BG + b + 1) * D],
                    start=(b == 0), stop=(b == BG - 1))
            ctx_sb = stp.tile([BG, D], bf, tag="ctx")
            nc.scalar.activation(ctx_sb[:], pctx[:], AF.Copy, scale=recip[:])
            # transpose context -> xT
            xT = stp.tile([128, NHC * BG], bf, tag=f"xT{g}")
            for cc in range(NHC):
                pt = ps_sm.tile([128, BG], bf, tag="sm")
                nc.tensor.transpose(pt[:], ctx_sb[:, cc * 128:(cc + 1) * 128],
                                    c_ident[:])
                nc.vector.tensor_copy(xT[:, cc * BG:(cc + 1) * BG], pt[:])
            # gates, quarter at a time: i, f, g, o
            hTp = hT_prev[g]
            gact = []
            for q in range(4):
                pg = ps_mm.tile([BG, H], f32, tag="mm32")
                for cc in range(NHC):
                    nc.tensor.matmul(
                        pg[:], xT[:, cc * BG:(cc + 1) * BG],
                        c_wih[:, cc * 4 * H + q * H:(cc * 4 + q + 1) * H],
                        start=(cc == 0), stop=False)
                nc.tensor.matmul(
                    pg[:], oh_cur[0][:, g * BG:(g + 1) * BG],
                    c_wihaug[:, q * H:(q + 1) * H], start=False, stop=False)
                for cc in range(NHC):
                    nc.tensor.matmul(
                        pg[:], hTp[:, cc * BG:(cc + 1) * BG],
                        c_whh[:, cc * 4 * H + q * H:(cc * 4 + q + 1) * H],
                        start=False, stop=(cc == NHC - 1))
                nm = ("sigi", "sigf", "tg", "sigo")[q]
                tile = stp.tile([BG, H], bf, tag=nm)
                if q == 2:
                    nc.scalar.activation(tile[:], pg[:], AF.Tanh)
                else:
                    nc.scalar.activation(tile[:], pg[:], AF.Tanh, scale=0.5)
                    nc.vector.tensor_scalar(tile[:], tile[:], 0.5, 0.5,
                                            OP.mult, OP.add)
                gact.append(tile)
            sigi, sigf, tg, sigo = gact
            # c = sigf*c + sigi*tg ; h = sigo*tanh(c)
            cs = c_st[g]
            t2 = stp.tile([BG, H], bf, tag="t2")
            nc.vector.tensor_tensor(cs[:], cs[:], sigf[:], OP.mult)
            nc.vector.tensor_tensor(t2[:], sigi[:], tg[:], OP.mult)
            nc.vector.tensor_tensor(cs[:], cs[:], t2[:], OP.add)
            th = stp.tile([BG, H], bf, tag="th")
            nc.scalar.activation(th[:], cs[:], AF.Tanh)
            h_bf = stp.tile([BG, H], bf, tag="h")
            nc.vector.tensor_tensor(h_bf[:], sigo[:], th[:], OP.mult)
            # transpose h -> hT
            hT = stp.tile([128, NHC * BG], bf, tag=f"hT{g}")
            for cc in range(NHC):
                pt = ps_sm.tile([128, BG], bf, tag="sm")
                nc.tensor.transpose(pt[:], h_bf[:, cc * 128:(cc + 1) * 128],
                                    c_ident[:])
                nc.vector.tensor_copy(hT[:, cc * BG:(cc + 1) * BG], pt[:])
            hT_prev[g] = hT
            # probs
            pp = ps_sm.tile([BG, C], f32, tag="sm")
            for cc in range(NHC):
                nc.tensor.matmul(pp[:], hT[:, cc * BG:(cc + 1) * BG],
                                 c_wgen[:, cc * C:(cc + 1) * C],
                                 start=(cc == 0), stop=False)
            nc.tensor.matmul(pp[:], c_onest[:], c_bgen[:], start=False, stop=True)
            po = stp.tile([BG, C], f32, tag="po")
            nc.scalar.copy(po[:], pp[:])
            nc.gpsimd.dma_start(out[g * BG:(g + 1) * BG, t * C:(t + 1) * C], po[:])

        for t in range(T):
            front(0, t)
            front(1, t)
            post(0, t)
            post(1, t)
    return nc


def _prep_inputs(batch_H, gt_label, w_i2h, w_h2h, b_h2h, w_score,
                 w_ih, w_hh, b_ih, b_hh, w_gen, b_gen):
    import ml_dtypes
    bf = ml_dtypes.bfloat16

    w_i2h = np.asarray(w_i2h, np.float32)
    w_h2h = np.asarray(w_h2h, np.float32)
    w_ih = np.asarray(w_ih, np.float32)
    w_hh = np.asarray(w_hh, np.float32)
    w_gen = np.asarray(w_gen, np.float32)
    wsc = np.asarray(w_score, np.float32)[0]
    b_h2h = np.asarray(b_h2h, np.float32)
    bias_ihhh = (np.asarray(b_ih, np.float32) + np.asarray(b_hh, np.float32))
    gt = np.asarray(gt_label).astype(np.int64)

    wi2ht = np.ascontiguousarray(w_i2h.T).astype(bf)
    wh2ht = np.ascontiguousarray(w_h2h.T).astype(bf)
    wiht = np.ascontiguousarray(w_ih[:, :D].T).astype(bf)
    wihaug = np.concatenate(
        [w_ih[:, D:D + C].T, bias_ihhh[None, :]], axis=0).astype(bf)
    whht = np.ascontiguousarray(w_hh.T).astype(bf)
    wgent = np.ascontiguousarray(w_gen.T).astype(bf)
    bgen = np.asarray(b_gen, np.float32)[None, :].astype(bf)
    bh2ht = np.ascontiguousarray(b_h2h.reshape(NHC, 128).T).astype(np.float32)

    wscst = np.zeros((128, NHC * 8 * 8), np.float32)
    for hc in range(NHC):
        for j in range(8):
            wscst[:, (hc * 8 + j) * 8 + j] = wsc[hc * 128:(hc + 1) * 128]
    wscst = wscst.astype(bf)

    ident = np.eye(BG, dtype=np.float32).astype(bf)
    ones1 = np.ones((128, 1), np.float32).astype(bf)
    onest = np.ones((1, BG), np.float32).astype(bf)

    in_maps = []
    for i in range(NCORES):
        sh = np.asarray(batch_H[i * BL:(i + 1) * BL], np.float32)  # [64,128,512]
        bht = np.ascontiguousarray(sh.transpose(2, 1, 0).reshape(D, M)).astype(bf)
        hsarr = np.ascontiguousarray(sh.transpose(1, 0, 2).reshape(S, BL * D)).astype(bf)
        oh = np.zeros((C + 1, T * BL), np.float32)
        gl = gt[i * BL:(i + 1) * BL]                               # [64, 26]
        for t in range(T):
            oh[gl[:, t], t * BL + np.arange(BL)] = 1.0
        oh[C, :] = 1.0
        in_maps.append({
            "bht": bht, "hs": hsarr, "wi2ht": wi2ht, "wh2ht": wh2ht,
            "wiht": wiht, "wihaug": wihaug, "whht": whht, "wgent": wgent,
            "bgen": bgen, "wscst": wscst, "onehot": oh.astype(bf),
            "bh2ht": bh2ht, "ident": ident, "ones1": ones1, "onest": onest,
        })
    return in_maps


LAST_RESULT = None


def _device_forward(in_maps, trace=False):
    from concourse.bass_utils import run_bass_kernel_spmd
    global LAST_RESULT
    if "nc" not in _cache:
        nc_ = _build_bass()
        nc_.finalize()
        _cache["nc"] = nc_
    res = run_bass_kernel_spmd(_cache["nc"], in_maps,
                               core_ids=list(range(NCORES)), trace=trace)
    LAST_RESULT = res
    outs = [r["out"].astype(np.float32).reshape(BL, T, C) for r in res.results]
    return np.concatenate(outs, axis=0)


def kernel(batch_H, gt_label, w_i2h, w_h2h, b_h2h, w_score,
           w_ih, w_hh, b_ih, b_hh, w_gen, b_gen):
    import os
    in_maps = _prep_inputs(batch_H, gt_label, w_i2h, w_h2h, b_h2h, w_score,
                           w_ih, w_hh, b_ih, b_hh, w_gen, b_gen)
    trace = bool(os.environ.get("KERNEL_TRACE"))
    return _device_forward(in_maps, trace=trace)
